# revision 1
# baseline (speedup 1.0000x reference)
"""Trainium2 Bass kernel for nn_BreakthroughSNN (predictive-coding SNN LM).

Strategy:
  - Replicate the sequential 128-step SNN recurrence on all 8 cores (the
    per-step tensors are tiny: (B=16, D=512)); shard the vocab dim of the
    final output projection (V=32000 -> 4000 per core). No collectives.
  - All per-step activations live in "T-layout" SBUF tiles [128, 64]:
    partition p, free index c*16+b  <->  element (d = c*128+p, b).
    This makes every elementwise op a dense 128-partition DVE op and makes
    matmul lhsT slices (activations stationary, K-chunk c = cols c*16..)
    free, while weights stream as the moving operand.
  - err = LN(relu(bu - pred)) feeding err @ inf_W.T is folded:
      err@W.T = istd * ((r*g)@W.T) - mu*istd*(g@W.T) + (b@W.T + inf_b)
    with r = relu(bu - pred).  g is folded into the weights on the host;
    G2 = g@W.T and B2 = b@W.T + inf_b are host-precomputed constants.
    So LN1 stats run concurrently with the matmul.
  - Top-down pass: bu=0 => relu(0-pred)=0 => err = LN(0) = ne_b (exact),
    so the top-down inf matmul input is the constant B2; only the gen
    matmuls remain in the top-down chain.
  - relu(relu(s) - p) = relu(s - p) for p in {0,1} removes the separate
    bottom-up relu.
  - LayerNorm partition-reductions via ones-matmul on PE; per-batch stats
    broadcast back across partitions via a ones[1,128] bcast-matmul.
"""

import os
import sys

sys.path.insert(0, "/opt/trn_rl_repo")

import numpy as np

import concourse.bass as bass
import concourse.bacc as bacc
from concourse import mybir
from concourse.bass_utils import run_bass_kernel_spmd
from concourse.tile import TileContext

F32 = mybir.dt.float32
Alu = mybir.AluOpType
Act = mybir.ActivationFunctionType

B, S, V, D, L = 16, 128, 32000, 512, 3
NCORES = 8
VS = V // NCORES  # 4000 vocab rows per core
DECAY = float(np.exp(-1.0 / 2.0))
THR = 1.0
EPS = 1e-5
C = D // 128  # 4 d-chunks
W64 = C * B  # 64: free width of a T-layout tile


def _wT_kmajor(W):
    """W: (D_out, D_in) -> SBUF layout [128, C*D_out], k-chunk-major.

    slice [:, k*D_out:(k+1)*D_out] is W.T[k*128:(k+1)*128, :]."""
    Dout, Din = W.shape
    return (
        np.ascontiguousarray(W.T)
        .reshape(Din // 128, 128, Dout)
        .transpose(1, 0, 2)
        .reshape(128, (Din // 128) * Dout)
        .astype(np.float32)
    )


def _vec_T(v):
    """v: (D,) -> T-layout tile [128, 64] (b-independent broadcast)."""
    t = v.reshape(C, 128).T  # [128, C]
    return np.ascontiguousarray(
        np.broadcast_to(t[:, :, None], (128, C, B)).reshape(128, W64)
    ).astype(np.float32)


def _tok_T(tok):
    """tok: (B,S,D) -> c-major [128, C*S*16]: [p, c*S*16 + t*16 + b]."""
    a = tok.transpose(2, 1, 0)  # [D, S, B]
    a = a.reshape(C, 128, S, B).transpose(1, 0, 2, 3)  # [128, C, S, B]
    return np.ascontiguousarray(a.reshape(128, C * S * B)).astype(np.float32)


def _build(nS):
    """Build the Bass program for nS recurrence steps."""
    nc = bacc.Bacc(None, target_bir_lowering=False)

    # ---- DRAM parameters ----
    d_tok = nc.declare_dram_parameter("tok_t", [128, nS * W64], F32, isOutput=False)
    d_encw = nc.declare_dram_parameter("encw_t", [128, C * D], F32, isOutput=False)
    d_genw = nc.declare_dram_parameter("genw_t", [L, 128, C * D], F32, isOutput=False)
    d_infw = nc.declare_dram_parameter("infw_t", [L, 128, C * D], F32, isOutput=False)
    d_outw = nc.declare_dram_parameter("outw_t", [128, C * VS], F32, isOutput=False)
    # small constants, stacked T-layout tiles along free dim [128, 15*64]:
    # 0..2 genb_T, 3..5 B2_T, 6..8 G2_T, 9..11 nsg_T, 12..14 nsb_T
    d_ctile = nc.declare_dram_parameter("ctiles", [128, 15 * W64], F32, isOutput=False)
    d_encb = nc.declare_dram_parameter("encb_c", [128, C], F32, isOutput=False)
    d_genbn = nc.declare_dram_parameter("genb_n", [1, L * D], F32, isOutput=False)
    d_ones = nc.declare_dram_parameter("ones_in", [128, 128], F32, isOutput=False)
    d_logits = nc.declare_dram_parameter("logits", [nS * B, VS], F32, isOutput=True)

    with TileContext(nc) as tc:
        with (
            tc.tile_pool(name="const", bufs=1) as cpool,
            tc.tile_pool(name="state", bufs=1) as spool,
            tc.tile_pool(name="work", bufs=3) as wpool,
            tc.tile_pool(name="stat", bufs=4) as stpool,
            tc.tile_pool(name="fin", bufs=4) as fpool,
            tc.tile_pool(name="owq", bufs=2) as owq,
            tc.tile_pool(name="psB", bufs=3, space="PSUM") as psB,
            tc.tile_pool(name="psS", bufs=5, space="PSUM") as psS,
        ):
            # ---- load constants / weights into SBUF ----
            encw = cpool.tile([128, C * D], F32, name="encw")
            nc.sync.dma_start(out=encw, in_=d_encw[:, :])
            genw = [cpool.tile([128, C * D], F32, tag=f"genw{j}", name=f"genw{j}") for j in range(L)]
            infw = [cpool.tile([128, C * D], F32, tag=f"infw{j}", name=f"infw{j}") for j in range(L)]
            for j in range(L):
                nc.sync.dma_start(out=genw[j], in_=d_genw[j])
                nc.sync.dma_start(out=infw[j], in_=d_infw[j])

            ct = cpool.tile([128, 15 * W64], F32, name="ct")
            nc.sync.dma_start(out=ct, in_=d_ctile[:, :])
            _cs = lambda i: ct[:, i * W64 : (i + 1) * W64]
            genbT = [_cs(j) for j in range(3)]
            B2T = [_cs(3 + j) for j in range(3)]
            G2T = [_cs(6 + j) for j in range(3)]
            nsgT = [_cs(9 + j) for j in range(3)]
            nsbT = [_cs(12 + j) for j in range(3)]

            encb = cpool.tile([128, C], F32, name="encb")
            nc.sync.dma_start(out=encb, in_=d_encb[:, :])
            gbn = cpool.tile([1, L * D], F32, name="gbn")
            nc.sync.dma_start(out=gbn, in_=d_genbn[:, :])
            genbN = [gbn[:, j * D : (j + 1) * D] for j in range(L)]

            onesin = cpool.tile([128, 128], F32, name="onesin")  # row0 = ones, used as [1,128]
            nc.sync.dma_start(out=onesin, in_=d_ones[:, :])
            ones_col = onesin[:, 0:1]  # [128,1] ones (lhsT for stat-mm)
            ones_row = onesin[0:1, :]  # [1,128] ones (lhsT for bcast-mm)
            ones_r16 = onesin[0:1, 0:16]  # [1,16] ones (lhsT for bias-mm)

            zeros = cpool.tile([128, W64], F32, name="zeros")
            nc.vector.memset(zeros, 0.0)
            epst = cpool.tile([1, 1], F32, name="epst")
            nc.vector.memset(epst, EPS)

            # tok tile doubles as td history (tok fully consumed in prologue)
            toktd = cpool.tile([128, nS * W64], F32, name="toktd")
            nc.sync.dma_start(out=toktd, in_=d_tok[:, :])
            xenc = cpool.tile([128, nS * W64], F32, name="xenc")

            # ---- persistent state tiles ----
            mem_enc = spool.tile([128, W64], F32, tag="mem_enc", name="mem_enc")
            mem_gen = [spool.tile([128, W64], F32, tag=f"mgen{j}", name=f"mgen{j}") for j in range(L)]
            mem_inf = [spool.tile([128, W64], F32, tag=f"minf{j}", name=f"minf{j}") for j in range(L)]
            states = [spool.tile([128, W64], F32, tag=f"st{j}", name=f"stt{j}") for j in range(L)]
            xgen = [spool.tile([128, W64], F32, tag=f"xg{j}", name=f"xg{j}") for j in range(L)]

            nc.vector.memset(mem_enc, 0.0)
            for j in range(L):
                nc.vector.memset(mem_gen[j], 0.0)
                nc.vector.memset(mem_inf[j], 0.0)
                nc.vector.memset(states[j], 0.0)
                # states0 = 0 -> x_gen(t=0) = gen_b
                nc.vector.tensor_copy(xgen[j], genbT[j])

            # identity for PE transpose
            ident = cpool.tile([128, 128], F32, name="ident")
            from concourse.masks import make_identity

            make_identity(nc, ident)

            # PE pre-touch of encw so the first real matmul carries only one
            # DMA-queue wait (PE LDWEIGHTS has a tiny sync-wait budget).
            ptch = psS.tile([16, 1], F32, tag="psmall", name="ptch")
            nc.tensor.transpose(ptch, encw[0:1, 0:16], ident[0:1, 0:1])

            # ---- prologue: x_enc = tok @ enc_W.T + enc_b, all steps ----
            # out chunk: x_enc.T rows m*128.. , cols = (t,b) block of 32 steps
            TB = nS * B  # cols per c-chunk in c-major tok/td layout
            xenc4 = xenc.rearrange("p (t c b) -> p c t b", c=C, b=B)
            nblk = nS * W64 // 512 if nS * W64 >= 512 else 1
            tblk = nS // nblk  # steps per 512-col block
            for m in range(C):
                for tc_i in range(nblk):
                    pE = psB.tile([128, 512], F32, tag="pbig", name="pE")
                    fd = tblk * B
                    for k in range(C):
                        nc.tensor.matmul(
                            pE[:, :fd],
                            encw[:, k * D + m * 128 : k * D + m * 128 + 128],
                            toktd[:, k * TB + tc_i * fd : k * TB + (tc_i + 1) * fd],
                            start=(k == 0),
                            stop=(k == C - 1),
                        )
                    nc.vector.tensor_scalar(
                        out=xenc4[:, m, tc_i * tblk : (tc_i + 1) * tblk, :],
                        in0=pE[:, :fd].rearrange("p (t b) -> p t b", b=B),
                        scalar1=encb[:, m : m + 1],
                        scalar2=None,
                        op0=Alu.add,
                    )

            # ---- helper closures ----
            def ln_stats(vtile, tag):
                """vtile: [128,128] with v in cols 0:64 and v^2 in 64:128.
                Returns pbc [128,32]: cols 0:16 = istd bcast, 16:32 = mu*istd."""
                pst = psS.tile([1, 128], F32, tag="psmall", name="pst")
                nc.tensor.matmul(pst, ones_col, vtile, start=True, stop=True)
                st = stpool.tile([1, 32], F32, tag=f"st_{tag}", name="st")
                # fold the 4 c-chunks: out[g,b] = sum_c pst[g*64 + c*16 + b]
                nc.vector.tensor_reduce(
                    out=st.rearrange("p (g b) -> p g b", g=2),
                    in_=pst.rearrange("p (g c b) -> p g b c", g=2, c=C),
                    axis=mybir.AxisListType.X,
                    op=Alu.add,
                )
                # st = [sum_v | sum_v2] -> [mu | E v2]
                nc.scalar.mul(st, st, 1.0 / D)
                mu2 = stpool.tile([1, 16], F32, tag=f"mu2_{tag}", name="mu2")
                nc.vector.tensor_mul(mu2, st[:, 0:16], st[:, 0:16])
                var = stpool.tile([1, 16], F32, tag=f"var_{tag}", name="var")
                nc.vector.tensor_sub(var, st[:, 16:32], mu2)
                bsrc = stpool.tile([1, 32], F32, tag=f"bsrc_{tag}", name="bsrc")
                # sd = sqrt(var + eps); istd = 1/sd
                nc.scalar.activation(
                    out=bsrc[:, 0:16], in_=var, func=Act.Sqrt, bias=epst[0:1, 0:1]
                )
                nc.vector.reciprocal(bsrc[:, 0:16], bsrc[:, 0:16])
                nc.vector.tensor_mul(bsrc[:, 16:32], st[:, 0:16], bsrc[:, 0:16])
                pbc = psS.tile([128, 32], F32, tag="psmall", name="pbc")
                nc.tensor.matmul(pbc, ones_row, bsrc, start=True, stop=True)
                return pbc

            def bc(pbc, lo):
                """[128,16] slice of pbc broadcast to [128,4,16]."""
                return pbc[:, None, lo : lo + 16].broadcast_to([128, C, 16])

            def as3(t):
                return t.rearrange("p (c b) -> p c b", c=C)

            def mm_TN(psum, lhsT64, w_sb, with_bias=None):
                """psum [16,512] = lhsT64.T @ W.T (+ bias row via bias-mm)."""
                first = True
                if with_bias is not None:
                    nc.tensor.matmul(psum, ones_r16, with_bias, start=True, stop=False)
                    first = False
                for k in range(C):
                    nc.tensor.matmul(
                        psum,
                        lhsT64[:, k * B : (k + 1) * B],
                        w_sb[:, k * D : (k + 1) * D],
                        start=first and k == 0,
                        stop=(k == C - 1),
                    )

            def transpose_NT2(xN, tag):
                pT = psS.tile([128, W64], F32, tag="psmall", name="pT")
                for c in range(C):
                    nc.tensor.transpose(
                        pT[:, c * B : (c + 1) * B],
                        xN[:, c * 128 : (c + 1) * 128],
                        ident[0:16, 0:16],
                    )
                return pT

            # ---- main recurrence ----
            for t in range(nS):
                # encoder LIF: bu0 = spike(mem_enc*dec + x_enc[t])
                nc.vector.scalar_tensor_tensor(
                    out=mem_enc,
                    in0=mem_enc,
                    scalar=DECAY,
                    in1=xenc[:, t * W64 : (t + 1) * W64],
                    op0=Alu.mult,
                    op1=Alu.add,
                )
                bu0 = wpool.tile([128, W64], F32, tag="bu0", name="bu0")
                nc.vector.tensor_scalar(
                    out=bu0, in0=mem_enc, scalar1=THR, scalar2=None, op0=Alu.is_ge
                )
                nc.vector.copy_predicated(mem_enc, bu0.bitcast(mybir.dt.int32), zeros)

                pred = [None] * L
                # ---- bottom-up ----
                for j in range(L):
                    # pred_j from xgen (computed in previous step's TD phase)
                    nc.vector.scalar_tensor_tensor(
                        out=mem_gen[j],
                        in0=mem_gen[j],
                        scalar=DECAY,
                        in1=xgen[j],
                        op0=Alu.mult,
                        op1=Alu.add,
                    )
                    pred[j] = wpool.tile([128, W64], F32, tag=f"pred{j}", name=f"pred{j}")
                    nc.vector.tensor_scalar(
                        out=pred[j],
                        in0=mem_gen[j],
                        scalar1=THR,
                        scalar2=None,
                        op0=Alu.is_ge,
                    )
                    nc.vector.copy_predicated(mem_gen[j], pred[j].bitcast(mybir.dt.int32), zeros)

                    # r = relu(bu - pred)   (bu = bu0 or states[j-1]; exact for j>0
                    # because relu(relu(s)-p) = relu(s-p) for p in {0,1})
                    bu_in = bu0 if j == 0 else states[j - 1]
                    r2 = wpool.tile([128, 2 * W64], F32, tag="r2", name="r2")
                    r = r2[:, 0:W64]
                    nc.vector.tensor_sub(r, bu_in, pred[j])
                    nc.vector.tensor_scalar(
                        out=r, in0=r, scalar1=0.0, scalar2=None, op0=Alu.max
                    )
                    # LN1 stats concurrent with matmul
                    nc.scalar.activation(
                        out=r2[:, W64 : 2 * W64], in_=r, func=Act.Square
                    )
                    pbc1 = ln_stats(r2, f"ln1_{j}")

                    # inf matmul on r (g folded into weights)
                    pM = psB.tile([16, 512], F32, tag="pbig", name="pM")
                    mm_TN(pM, r, infw[j])
                    xN = wpool.tile([16, 512], F32, tag="xN", name="xN")
                    nc.scalar.copy(xN, pM)
                    pT = transpose_NT2(xN, "inf")
                    rT = wpool.tile([128, W64], F32, tag="rT", name="rT")
                    nc.vector.tensor_copy(rT, pT)

                    # su = spike(mem_inf*dec + istd*rT + B2 - mu*istd*G2)
                    base = wpool.tile([128, W64], F32, tag="base", name="base")
                    nc.vector.scalar_tensor_tensor(
                        out=base,
                        in0=mem_inf[j],
                        scalar=DECAY,
                        in1=B2T[j],
                        op0=Alu.mult,
                        op1=Alu.add,
                    )
                    gsc = wpool.tile([128, W64], F32, tag="gsc", name="gsc")
                    nc.vector.tensor_mul(as3(gsc), as3(G2T[j]), bc(pbc1, 16))
                    nc.vector.tensor_sub(base, base, gsc)
                    xsc = wpool.tile([128, W64], F32, tag="xsc", name="xsc")
                    nc.vector.tensor_mul(as3(xsc), as3(rT), bc(pbc1, 0))
                    nc.vector.tensor_add(mem_inf[j], xsc, base)
                    su = wpool.tile([128, W64], F32, tag="su", name="su")
                    nc.vector.tensor_scalar(
                        out=su, in0=mem_inf[j], scalar1=THR, scalar2=None, op0=Alu.is_ge
                    )
                    nc.vector.copy_predicated(mem_inf[j], su.bitcast(mybir.dt.int32), zeros)

                    # state' = LN_ns(state + su)
                    w2 = wpool.tile([128, 2 * W64], F32, tag="w2", name="w2")
                    wv = w2[:, 0:W64]
                    nc.vector.tensor_add(wv, states[j], su)
                    nc.scalar.activation(
                        out=w2[:, W64 : 2 * W64], in_=wv, func=Act.Square
                    )
                    pbc2 = ln_stats(w2, f"ln2_{j}")
                    t1 = wpool.tile([128, W64], F32, tag="t1", name="t1")
                    nc.vector.tensor_mul(as3(t1), as3(wv), bc(pbc2, 0))
                    nc.vector.tensor_tensor(
                        out=as3(t1), in0=as3(t1), in1=bc(pbc2, 16), op=Alu.subtract
                    )
                    nc.vector.tensor_mul(t1, t1, nsgT[j])
                    nc.vector.tensor_add(states[j], t1, nsbT[j])

                # ---- top-down ----
                td_in = states[L - 1]
                for j in reversed(range(L)):
                    # gen matmul for TD chain ...
                    pTD = psB.tile([16, 512], F32, tag="pbig", name="pTD")
                    mm_TN(pTD, td_in, genw[j], with_bias=genbN[j])
                    # ... and gen matmul for next step's pred (same weights)
                    pXG = psB.tile([16, 512], F32, tag="pbig", name="pXG")
                    mm_TN(pXG, states[j], genw[j], with_bias=genbN[j])

                    tdN = wpool.tile([16, 512], F32, tag="tdN", name="tdN")
                    nc.scalar.copy(tdN, pTD)
                    pTDt = transpose_NT2(tdN, "td")
                    nc.vector.scalar_tensor_tensor(
                        out=mem_gen[j],
                        in0=mem_gen[j],
                        scalar=DECAY,
                        in1=pTDt,
                        op0=Alu.mult,
                        op1=Alu.add,
                    )
                    if j > 0:
                        p_j = wpool.tile([128, W64], F32, tag=f"p_td{j}", name="p_j")
                        p_out = p_j
                        in0v = mem_gen[j]
                    else:
                        # write td_t (binary) into c-major td history
                        p_out = toktd.rearrange("p (c t b) -> p c t b", c=C, b=B)[
                            :, :, t, :
                        ]
                        in0v = mem_gen[j].rearrange("p (c b) -> p c b", c=C)
                    nc.vector.tensor_scalar(
                        out=p_out,
                        in0=in0v,
                        scalar1=THR,
                        scalar2=None,
                        op0=Alu.is_ge,
                    )
                    if j > 0:
                        nc.vector.copy_predicated(
                            mem_gen[j], p_out.bitcast(mybir.dt.int32), zeros
                        )
                        td_in = p_j
                    else:
                        nc.vector.copy_predicated(
                            in0v,
                            p_out.bitcast(mybir.dt.int32),
                            zeros.rearrange("p (c b) -> p c b", c=C),
                        )

                    # xgen for next step
                    xgN = wpool.tile([16, 512], F32, tag="xgN", name="xgN")
                    nc.scalar.copy(xgN, pXG)
                    pXGt = transpose_NT2(xgN, "xg")
                    nc.vector.tensor_copy(xgen[j], pXGt)

                    # TD mem_inf update with constant input B2
                    nc.vector.scalar_tensor_tensor(
                        out=mem_inf[j],
                        in0=mem_inf[j],
                        scalar=DECAY,
                        in1=B2T[j],
                        op0=Alu.mult,
                        op1=Alu.add,
                    )
                    si = wpool.tile([128, W64], F32, tag="si", name="si")
                    nc.vector.tensor_scalar(
                        out=si, in0=mem_inf[j], scalar1=THR, scalar2=None, op0=Alu.is_ge
                    )
                    nc.vector.copy_predicated(mem_inf[j], si.bitcast(mybir.dt.int32), zeros)

            # ---- final projection: logits = td @ out_W_shard.T ----
            TB = nS * B
            n_mb = (nS * B) // 128 if nS * B >= 128 else 1
            t_mb = nS // n_mb  # steps per M-block
            NB = VS // 500  # 8 chunks of 500
            # outw streamed per 500-vocab-col chunk: [128, C*500], k-major
            for nb in range(NB):
                outwq = owq.tile([128, C * 500], F32, tag="outwq", name="outwq")
                nc.sync.dma_start(
                    out=outwq, in_=d_outw[:, nb * C * 500 : (nb + 1) * C * 500]
                )
                if True:
                    for mb in range(n_mb):
                        pf = psB.tile([128, 512], F32, tag="pbig", name="pf")
                        fd = t_mb * B
                        for k in range(C):
                            nc.tensor.matmul(
                                pf[:fd, 0:500],
                                toktd[:, k * TB + mb * fd : k * TB + (mb + 1) * fd],
                                outwq[:, k * 500 : (k + 1) * 500],
                                start=(k == 0),
                                stop=(k == C - 1),
                            )
                        fo = fpool.tile([128, 500], F32, tag="fo", name="fo")
                        if mb % 2 == 0:
                            nc.vector.tensor_copy(fo[:fd], pf[:fd, 0:500])
                        else:
                            nc.scalar.copy(fo[:fd], pf[:fd, 0:500])
                        nc.sync.dma_start(
                            out=d_logits[
                                mb * fd : (mb + 1) * fd, nb * 500 : (nb + 1) * 500
                            ],
                            in_=fo[:fd],
                        )

    return nc


_CACHE = {}
TRACE = False
LAST_RESULTS = None


def _get_program(nS):
    if nS not in _CACHE:
        nc = _build(nS)
        nc.finalize()
        _CACHE[nS] = nc
    return _CACHE[nS]


def kernel(**inputs):
    input_ids = np.asarray(inputs["input_ids"])
    emb = np.asarray(inputs["emb"], np.float32)
    enc_W = np.asarray(inputs["enc_W"], np.float32)
    enc_b = np.asarray(inputs["enc_b"], np.float32)
    gen_W = np.asarray(inputs["gen_W"], np.float32)
    gen_b = np.asarray(inputs["gen_b"], np.float32)
    inf_W = np.asarray(inputs["inf_W"], np.float32)
    inf_b = np.asarray(inputs["inf_b"], np.float32)
    ns_g = np.asarray(inputs["ns_g"], np.float32)
    ns_b = np.asarray(inputs["ns_b"], np.float32)
    ne_g = np.asarray(inputs["ne_g"], np.float32)
    ne_b = np.asarray(inputs["ne_b"], np.float32)
    out_W = np.asarray(inputs["out_W"], np.float32)
    out_b = np.asarray(inputs["out_b"], np.float32)

    nB, nS = input_ids.shape
    assert (nB, nS) == (B, S), (nB, nS)

    tok = emb[input_ids]  # (B, S, D) host gather

    # host-side constant folding
    ctiles = np.zeros((15, 128, W64), np.float32)
    genw_t = np.zeros((L, 128, C * D), np.float32)
    infw_t = np.zeros((L, 128, C * D), np.float32)
    genb_n = np.zeros((1, L * D), np.float32)
    for j in range(L):
        ctiles[j] = _vec_T(gen_b[j])
        B2 = ne_b[j] @ inf_W[j].T + inf_b[j]
        G2 = ne_g[j] @ inf_W[j].T
        ctiles[3 + j] = _vec_T(B2)
        ctiles[6 + j] = _vec_T(G2)
        ctiles[9 + j] = _vec_T(ns_g[j])
        ctiles[12 + j] = _vec_T(ns_b[j])
        genw_t[j] = _wT_kmajor(gen_W[j])
        infw_t[j] = _wT_kmajor(inf_W[j] * ne_g[j][None, :])
        genb_n[0, j * D : (j + 1) * D] = gen_b[j]
    ctiles_packed = np.ascontiguousarray(
        ctiles.transpose(1, 0, 2).reshape(128, 15 * W64)
    )

    shared = {
        "tok_t": _tok_T(tok),
        "encw_t": _wT_kmajor(enc_W),
        "genw_t": genw_t,
        "infw_t": infw_t,
        "ctiles": ctiles_packed,
        "encb_c": np.ascontiguousarray(enc_b.reshape(C, 128).T).astype(np.float32),
        "genb_n": genb_n,
        "ones_in": np.ones((128, 128), np.float32),
    }

    nc = _get_program(S)
    in_maps = []
    for i in range(NCORES):
        m = dict(shared)
        shard = out_W[i * VS : (i + 1) * VS]
        m["outw_t"] = np.concatenate(
            [_wT_kmajor(shard[q * 500 : (q + 1) * 500]) for q in range(8)], axis=1
        )
        in_maps.append(m)

    global LAST_RESULTS
    if TRACE:
        res = run_bass_kernel_spmd(
            nc, in_maps, list(range(NCORES)), trace=True
        )
    else:
        res = run_bass_kernel_spmd(nc, in_maps, list(range(NCORES)))
    LAST_RESULTS = res
    shards = []
    for i in range(NCORES):
        lg = res.results[i]["logits"].reshape(S, B, VS).transpose(1, 0, 2)
        shards.append(lg)
    logits = np.concatenate(shards, axis=2)  # (B, S, V)
    logits = logits + out_b[None, None, :]
    return logits.astype(np.float32)


if __name__ == "__main__":
    pass

